# revision 1
# baseline (speedup 1.0000x reference)
import sys

sys.path.insert(0, "/opt/trn_rl_repo")

import numpy as np

B, C, H, W = 8, 81, 96, 320
F = H * W
NBOX, NCAND = 16, 17
ALPHA = 0.25
DEPTH_MIN, DEPTH_MAX, NUM_BINS = 0.001, 60.0, 80

STRIDE = 32.0
OFF = 16.0
BIG = 1024.0
KCOL = W * NCAND
QCOL = KCOL // 4

SUB = 16
NSUBC = H // SUB
GRP = 32
SPG = GRP // SUB

_PROG = None


def _build_program():
    from concourse import bacc, tile, mybir

    f32 = mybir.dt.float32
    bf16 = mybir.dt.bfloat16
    i32 = mybir.dt.int32
    AF = mybir.ActivationFunctionType
    OP = mybir.AluOpType

    nc = bacc.Bacc(
        "TRN2",
        target_bir_lowering=False,
        debug=False,
        enable_asserts=False,
    )

    f8 = mybir.dt.float8e4
    L = nc.dram_tensor("logits", [C, F], f8, kind="ExternalInput")
    lgat_d = nc.dram_tensor("lgat", [H, KCOL], f8, kind="ExternalInput")
    bdc_d = nc.dram_tensor("bdc", [NCAND + 1, KCOL], bf16, kind="ExternalInput")
    w18_d = nc.dram_tensor("w18", [NCAND + 1, H], bf16, kind="ExternalInput")
    diag32_d = nc.dram_tensor("diag32", [C, GRP * GRP], f8, kind="ExternalInput")
    out_d = nc.dram_tensor("out", [1, 1], f32, kind="ExternalOutput")

    import os

    dbg = os.environ.get("KERNEL_DEBUG") == "1"
    if dbg:
        dbg_m = nc.dram_tensor("dbg_m", [H, W], f32, kind="ExternalOutput")
        dbg_s = nc.dram_tensor("dbg_s", [H, W], f32, kind="ExternalOutput")

    PIX = SUB * W

    with tile.TileContext(nc) as tc:
        with (
            tc.tile_pool(name="persist", bufs=1) as pp,
            tc.tile_pool(name="lc", bufs=1) as lcp,
            tc.tile_pool(name="ec", bufs=1) as ecp,
            tc.tile_pool(name="enc", bufs=2) as ep,
            tc.tile_pool(name="spsum", bufs=1, space="PSUM") as sp,
            tc.tile_pool(name="ppsum", bufs=2, space="PSUM") as qp,
            tc.tile_pool(name="opsum", bufs=1, space="PSUM") as op_,
        ):
            bounds = [0, 8, 16, 32, 48, 64, 72, 80, 88, 96]
            spans = list(zip(bounds[:-1], bounds[1:]))
            lcs = []
            for v0, v1 in spans:
                lc = lcp.tile([C, (v1 - v0) * W], f8, tag=f"lc{v0}")
                lcs.append(lc)
            diag32 = pp.tile([C, GRP * GRP], f8)
            nc.sync.dma_start(diag32[:], diag32_d[:])
            bdc = pp.tile([NCAND + 1, KCOL], bf16)
            w18 = pp.tile([NCAND + 1, H], bf16)
            ones96 = pp.tile([H, 1], bf16)
            nc.vector.memset(ones96[:], 1.0)
            RING = {0: "sync", 1: "gpsimd", 2: "sync", 3: "gpsimd", 4: "sync",
                    5: "gpsimd", 6: "sync", 7: "gpsimd", 8: "sync"}
            lgat = pp.tile([H, KCOL], f8)
            for i, (v0, v1) in enumerate(spans):
                ring = getattr(nc, RING[i])
                ring.dma_start(lcs[i][:], L[:, v0 * W : v1 * W])
                if i == 3:
                    nc.gpsimd.dma_start(lgat[:], lgat_d[:])
                if i == 4:
                    nc.sync.dma_start(bdc[:], bdc_d[:])
                    nc.sync.dma_start(w18[:], w18_d[:])

            s_ps = sp.tile([H, W], f32)
            mstar = pp.tile([H, W], f32)

            def pen_quarter(q):
                pen = qp.tile([H, QCOL], f32)
                for c0, cn in ((0, 512), (512, 512), (1024, QCOL - 1024)):
                    nc.tensor.matmul(
                        pen[:, c0 : c0 + cn],
                        w18[:],
                        bdc[:, q * QCOL + c0 : q * QCOL + c0 + cn],
                        start=True,
                        stop=True,
                    )
                enc = ep.tile([H, QCOL], f32, tag="enc")
                nc.vector.tensor_tensor(
                    enc[:], lgat[:, q * QCOL : (q + 1) * QCOL], pen[:], op=OP.add
                )
                nc.vector.tensor_reduce(
                    mstar[:, q * (W // 4) : (q + 1) * (W // 4)],
                    enc[:].rearrange("v (u k) -> v u k", k=NCAND),
                    axis=mybir.AxisListType.X,
                    op=OP.min,
                )

            pen_at = {32: 0, 48: 1, 64: 2, 80: 3}
            for i, (v0, v1) in enumerate(spans):
                ec = ecp.tile([C, (v1 - v0) * W], bf16, tag=f"ec{v0}")
                nc.scalar.activation(ec[:], lcs[i][:], AF.Exp)
                for r, v in enumerate(range(v0, v1)):
                    g, rr = v // GRP, v % GRP
                    nc.tensor.matmul(
                        s_ps[GRP * g : GRP * (g + 1), :],
                        diag32[:, GRP * rr : GRP * (rr + 1)],
                        ec[:, r * W : (r + 1) * W],
                        start=(rr == 0),
                        stop=(rr == GRP - 1),
                    )
                if v1 in pen_at:
                    pen_quarter(pen_at[v1])

            r_i = pp.tile([H, W], i32)
            nc.vector.tensor_scalar(
                r_i[:], mstar[:], 1.0 / STRIDE, -0.25, op0=OP.mult, op1=OP.add
            )
            r_f = pp.tile([H, W], f32)
            nc.vector.tensor_copy(r_f[:], r_i[:])
            lam = pp.tile([H, W], f32)
            nc.vector.scalar_tensor_tensor(
                lam[:], r_f[:], -STRIDE, mstar[:], op0=OP.mult, op1=OP.add
            )
            wgt = pp.tile([H, W], f32)
            nc.vector.tensor_scalar(
                wgt[:], mstar[:], STRIDE * NBOX, 12.0, op0=OP.is_lt, op1=OP.mult
            )

            lmo = pp.tile([H, W], f32)
            nc.vector.tensor_scalar(
                lmo[:], lam[:], 1.0, -OFF, op0=OP.mult, op1=OP.add
            )
            e_lam = pp.tile([H, W], f32)
            nc.scalar.activation(e_lam[:], lmo[:], AF.Exp)
            ln_s = pp.tile([H, W], f32)
            nc.scalar.activation(ln_s[:], s_ps[:], AF.Ln)
            rs = pp.tile([H, W], f32)
            nc.vector.reciprocal_approx_fast(rs[:], s_ps[:])
            p = pp.tile([H, W], f32)
            nc.vector.tensor_tensor(p[:], e_lam[:], rs[:], op=OP.mult)
            logp = pp.tile([H, W], f32)
            nc.vector.tensor_tensor(logp[:], lmo[:], ln_s[:], op=OP.subtract)
            omm = pp.tile([H, W], f32)
            nc.vector.tensor_scalar(
                omm[:], p[:], -1.0, 1.0, op0=OP.mult, op1=OP.add
            )
            sq = pp.tile([H, W], f32)
            nc.vector.tensor_tensor(sq[:], omm[:], omm[:], op=OP.mult)
            t1 = pp.tile([H, W], f32)
            nc.vector.tensor_tensor(t1[:], sq[:], logp[:], op=OP.mult)
            wl = pp.tile([H, W], bf16)
            nc.vector.scalar_tensor_tensor(
                wl[:], wgt[:], 1.0, t1[:], op0=OP.add, op1=OP.mult
            )
            osum_ps = op_.tile([1, W], f32)
            nc.tensor.matmul(osum_ps[:], ones96[:], wl[:], start=True, stop=True)
            osum = pp.tile([1, 1], f32)
            nc.vector.tensor_reduce(
                osum[:], osum_ps[:], axis=mybir.AxisListType.X, op=OP.add
            )
            nc.sync.dma_start(out_d[:], osum[:])
            if dbg:
                nc.sync.dma_start(dbg_m[:], mstar[:])
                dbg_sb = pp.tile([H, W], f32)
                nc.vector.tensor_copy(dbg_sb[:], s_ps[:])
                nc.sync.dma_start(dbg_s[:], dbg_sb[:])

    nc.compile()
    return nc


def _bin_of(depth):
    d = np.float32(depth)
    bin_size = np.float32(2.0 * (DEPTH_MAX - DEPTH_MIN) / (NUM_BINS * (1 + NUM_BINS)))
    idx = np.float32(-0.5) + np.float32(0.5) * np.sqrt(
        np.float32(1.0) + np.float32(8.0) * (d - np.float32(DEPTH_MIN)) / bin_size
    )
    bad = (idx < 0) | (idx > NUM_BINS) | ~np.isfinite(idx)
    idx = np.where(bad, np.float32(NUM_BINS), idx)
    return np.rint(idx).astype(np.int32)


def _host_prep(depth_logits, gt_boxes2d, num_gt_per_img, gt_center_depth):
    import ml_dtypes

    n = int(num_gt_per_img)
    boxes = np.asarray(gt_boxes2d, np.float32).reshape(B, n, 4)
    depths = np.asarray(gt_center_depth, np.float32).reshape(B, n)
    logits_f32 = np.asarray(depth_logits, np.float32).reshape(B, C, F)
    logits_f8 = logits_f32.astype(ml_dtypes.float8_e4m3fn)

    diag32 = np.zeros((C, GRP * GRP), np.float32)
    for r in range(GRP):
        diag32[:, GRP * r + r] = 1.0
    diag32 = diag32.astype(ml_dtypes.float8_e4m3fn)

    us = np.arange(W, dtype=np.float32)
    vs = np.arange(H, dtype=np.float32)
    ks = np.arange(NCAND, dtype=np.float32)
    kk = np.arange(NCAND)
    bd_rows = np.zeros((NCAND, KCOL), np.float32)
    for u in range(W):
        bd_rows[kk, u * NCAND + kk] = 1.0

    in_maps = []
    for i in range(B):
        bins = _bin_of(depths[i])
        order = np.argsort(bins, kind="stable")
        u1 = np.floor(boxes[i, order, 0])
        v1 = np.floor(boxes[i, order, 1])
        u2 = np.ceil(boxes[i, order, 2])
        v2 = np.ceil(boxes[i, order, 3])
        cand = np.concatenate([bins[order], [NUM_BINS]]).astype(np.int32)
        u1c = np.concatenate([u1, [0.0]]).astype(np.float32)
        u2c = np.concatenate([u2, [W]]).astype(np.float32)
        v1c = np.concatenate([v1, [0.0]]).astype(np.float32)
        v2c = np.concatenate([v2, [H]]).astype(np.float32)

        colm = ((us[None] >= u1c[:, None]) & (us[None] < u2c[:, None])).astype(
            np.float32
        )
        rowm = ((vs[None] >= v1c[:, None]) & (vs[None] < v2c[:, None])).astype(
            np.float32
        )
        cflat = (
            -BIG * colm + (2.0 * BIG + STRIDE * ks[:, None] + OFF)
        ).T.reshape(-1)
        bdc = np.concatenate([bd_rows, cflat[None, :]], axis=0).astype(
            ml_dtypes.bfloat16
        )
        w18 = np.concatenate(
            [-BIG * rowm, np.ones((1, H), np.float32)], axis=0
        ).astype(ml_dtypes.bfloat16)

        lgat = np.ascontiguousarray(
            logits_f8[i][cand].reshape(NCAND, H, W).transpose(1, 2, 0)
        ).reshape(H, KCOL)

        in_maps.append(
            {
                "logits": logits_f8[i],
                "lgat": lgat,
                "bdc": bdc,
                "w18": w18,
                "diag32": diag32,
            }
        )
    return in_maps


def get_program():
    global _PROG
    if _PROG is None:
        _PROG = _build_program()
    return _PROG


def kernel(depth_logits, gt_boxes2d, num_gt_per_img, gt_center_depth, _trace=False):
    from concourse import bass_utils

    nc = get_program()
    in_maps = _host_prep(depth_logits, gt_boxes2d, num_gt_per_img, gt_center_depth)
    res = bass_utils.run_bass_kernel_spmd(
        nc, in_maps, core_ids=list(range(B)), trace=_trace
    )
    total = np.float64(0.0)
    for r in res.results:
        total += np.float64(r["out"].astype(np.float64).sum())
    loss = np.float32(-ALPHA * total / (B * H * W))
    if _trace:
        kernel._last_results = res
    return np.asarray(loss, dtype=np.float32)



# revision 10
# speedup vs baseline: 1.1624x; 1.1624x over previous
import sys

sys.path.insert(0, "/opt/trn_rl_repo")

import numpy as np

B, C, H, W = 8, 81, 96, 320
F = H * W
NBOX = 16
ALPHA = 0.25
DEPTH_MIN, DEPTH_MAX, NUM_BINS = 0.001, 60.0, 80

STRIDE = 32.0
OFF = 16.0
BIG = 1024.0
K = 7
KCOL = W * K

VC = 32
FC = VC * W
PXROWS = H - VC
HPX = W // 2
PXCOL = PXROWS * W * C // 128

NCH = 4
NPX = 4
CHCOL = FC // NCH
PXCHUNK = PXCOL // NPX
GPC = PXCHUNK // C

CH_SPLIT = (1664, 576, 320)
PX_SPLIT_G = (22, 13, 5)

A_SCH = 1024.0 * np.float64(np.log2(np.e))
B_SCH = 15360.0 - 60.0
LN2 = float(np.log(2.0))
SLN = 0.0435

_PROG = None


def _build_program():
    from concourse import bacc, tile, mybir

    f32 = mybir.dt.float32
    f16 = mybir.dt.float16
    bf16 = mybir.dt.bfloat16
    i16 = mybir.dt.int16
    i32 = mybir.dt.int32
    f8 = mybir.dt.float8e4
    AF = mybir.ActivationFunctionType
    OP = mybir.AluOpType

    nc = bacc.Bacc(
        "TRN2",
        target_bir_lowering=False,
        debug=False,
        enable_asserts=False,
    )

    lch_d = nc.dram_tensor("lch", [C, FC], f8, kind="ExternalInput")
    lpx_d = nc.dram_tensor("lpx", [128, PXCOL], f8, kind="ExternalInput")
    lgat_d = nc.dram_tensor("lgat", [H, KCOL], f8, kind="ExternalInput")
    bdcw_d = nc.dram_tensor("bdcw", [18, KCOL + H], bf16, kind="ExternalInput")
    diag32_d = nc.dram_tensor("diag32", [C, VC * VC], f8, kind="ExternalInput")
    ident96_d = nc.dram_tensor("ident96", [H, H], f8, kind="ExternalInput")
    out_d = nc.dram_tensor("out", [1, 1], f32, kind="ExternalOutput")

    import os

    dbg = os.environ.get("KERNEL_DEBUG") == "1"
    if dbg:
        dbg_m = nc.dram_tensor("dbg_m", [H, W], f32, kind="ExternalOutput")
        dbg_s = nc.dram_tensor("dbg_s", [H, W], f32, kind="ExternalOutput")

    with tile.TileContext(nc) as tc:
        with (
            tc.tile_pool(name="persist", bufs=1) as pp,
            tc.tile_pool(name="tree", bufs=2) as tp,
            tc.tile_pool(name="spsum", bufs=1, space="PSUM") as sp,
            tc.tile_pool(name="ppsum", bufs=1, space="PSUM") as qp,
            tc.tile_pool(name="opsum", bufs=1, space="PSUM") as op_,
        ):
            lch = pp.tile([C, FC], f8)
            lpx = pp.tile([128, PXCOL], f8)
            ec = pp.tile([C, FC], f16)
            epx = pp.tile([128, PXCOL], f16)
            lgat = pp.tile([H, KCOL], f8)
            bdcw = pp.tile([18, KCOL + H], bf16)
            diag32 = pp.tile([C, VC * VC], f8)
            ident96 = pp.tile([H, H], f8)
            s_px = pp.tile([128, HPX], f32)
            s_full = pp.tile([H, W], f32)
            ones96 = pp.tile([H, 1], bf16)
            nc.vector.memset(ones96[:], 1.0)

            for k in range(NCH):
                nc.sync.dma_start(
                    lch[:, k * CHCOL : (k + 1) * CHCOL],
                    lch_d[:, k * CHCOL : (k + 1) * CHCOL],
                )
            for k in range(2):
                nc.scalar.dma_start(
                    lpx[:, k * PXCHUNK : (k + 1) * PXCHUNK],
                    lpx_d[:, k * PXCHUNK : (k + 1) * PXCHUNK],
                )
            nc.gpsimd.dma_start(diag32[:], diag32_d[:])
            nc.gpsimd.dma_start(bdcw[:], bdcw_d[:])
            nc.gpsimd.dma_start(ident96[:], ident96_d[:])
            nc.gpsimd.dma_start(lgat[:], lgat_d[:])
            for k in range(2, NPX):
                nc.gpsimd.dma_start(
                    lpx[:, k * PXCHUNK : (k + 1) * PXCHUNK],
                    lpx_d[:, k * PXCHUNK : (k + 1) * PXCHUNK],
                )

            s_ps = sp.tile([VC, W], f32)
            pen = qp.tile([H, KCOL], f32)
            mstar = pp.tile([H, W], f32)

            w18 = bdcw[:, KCOL : KCOL + H]
            for c0 in range(0, KCOL, 512):
                cn = min(512, KCOL - c0)
                nc.tensor.matmul(
                    pen[:, c0 : c0 + cn],
                    w18,
                    bdcw[:, c0 : c0 + cn],
                    start=True,
                    stop=False,
                )

            def exp_split(dst, src, base, widths):
                a, p, d = widths
                c0 = base
                nc.scalar.activation(dst[:, c0 : c0 + a], src[:, c0 : c0 + a], AF.Exp)
                c0 += a
                nc.gpsimd.tensor_scalar(
                    dst[:, c0 : c0 + p].bitcast(i16),
                    src[:, c0 : c0 + p],
                    A_SCH,
                    B_SCH,
                    op0=OP.mult,
                    op1=OP.add,
                )
                c0 += p
                nc.vector.tensor_scalar(
                    dst[:, c0 : c0 + d].bitcast(i16),
                    src[:, c0 : c0 + d],
                    A_SCH,
                    B_SCH,
                    op0=OP.mult,
                    op1=OP.add,
                )

            rows_per_chunk = VC // NCH
            for k in range(max(NCH, NPX)):
                if k < NCH:
                    exp_split(ec, lch, k * CHCOL, CH_SPLIT)
                    for r in range(rows_per_chunk):
                        v = k * rows_per_chunk + r
                        nc.tensor.matmul(
                            s_ps[:],
                            diag32[:, VC * v : VC * (v + 1)],
                            ec[:, v * W : (v + 1) * W],
                            start=(v == 0),
                            stop=(v == VC - 1),
                        )
                if k < NPX:
                    base = k * PXCHUNK
                    ga, gp, gd = PX_SPLIT_G
                    exp_split(
                        epx, lpx, base, (ga * C, gp * C, gd * C)
                    )
                    g3 = epx[:, base : base + PXCHUNK].rearrange(
                        "p (g c) -> p g c", c=C
                    )
                    tb = tp.tile([128, GPC * 40], f16, tag="tb")
                    tc_ = tp.tile([128, GPC * 20], f16, tag="tc")
                    td = tp.tile([128, GPC * 10], f16, tag="td")
                    te = tp.tile([128, GPC * 5], f16, tag="te")
                    tf = tp.tile([128, GPC * 2], f16, tag="tf")
                    tg = tp.tile([128, GPC], f16, tag="tg")
                    th = tp.tile([128, GPC], f16, tag="th")
                    b3 = tb[:].rearrange("p (g c) -> p g c", c=40)
                    c3 = tc_[:].rearrange("p (g c) -> p g c", c=20)
                    d3 = td[:].rearrange("p (g c) -> p g c", c=10)
                    e3 = te[:].rearrange("p (g c) -> p g c", c=5)
                    f3 = tf[:].rearrange("p (g c) -> p g c", c=2)
                    tg3 = tg[:].rearrange("p (g c) -> p g c", c=1)
                    th3 = th[:].rearrange("p (g c) -> p g c", c=1)
                    so3 = s_px[:, k * GPC : (k + 1) * GPC].rearrange(
                        "p (g c) -> p g c", c=1
                    )
                    TT = nc.vector.tensor_tensor
                    TT(b3, g3[:, :, 0:40], g3[:, :, 40:80], op=OP.add)
                    TT(c3, b3[:, :, 0:20], b3[:, :, 20:40], op=OP.add)
                    TT(d3, c3[:, :, 0:10], c3[:, :, 10:20], op=OP.add)
                    TT(e3, d3[:, :, 0:5], d3[:, :, 5:10], op=OP.add)
                    TT(f3, e3[:, :, 0:2], e3[:, :, 2:4], op=OP.add)
                    TT(tg3, f3[:, :, 0:1], f3[:, :, 1:2], op=OP.add)
                    TT(th3, tg3, g3[:, :, 80:81], op=OP.add)
                    TT(so3, th3, e3[:, :, 4:5], op=OP.add)
                if k == 1:
                    for c0 in range(0, KCOL, 512):
                        cn = min(512, KCOL - c0)
                        nc.tensor.matmul(
                            pen[:, c0 : c0 + cn],
                            ident96[:],
                            lgat[:, c0 : c0 + cn],
                            start=False,
                            stop=True,
                        )

            nc.vector.tensor_reduce(
                mstar[:],
                pen[:].rearrange("v (u k) -> v u k", k=K),
                axis=mybir.AxisListType.X,
                op=OP.min,
            )
            r_i = pp.tile([H, W], i32)
            nc.gpsimd.tensor_scalar(
                r_i[:], mstar[:], 1.0 / STRIDE, -0.25, op0=OP.mult, op1=OP.add
            )
            r_f = pp.tile([H, W], f32)
            nc.gpsimd.tensor_copy(r_f[:], r_i[:])
            lam = pp.tile([H, W], f32)
            nc.vector.scalar_tensor_tensor(
                lam[:], r_f[:], -STRIDE, mstar[:], op0=OP.mult, op1=OP.add
            )
            wgt = pp.tile([H, W], f32)
            nc.gpsimd.tensor_scalar(
                wgt[:], mstar[:], STRIDE * NBOX, 12.0, op0=OP.is_lt, op1=OP.mult
            )
            lmo = pp.tile([H, W], f32)
            nc.gpsimd.tensor_scalar(
                lmo[:], lam[:], 1.0, -OFF, op0=OP.mult, op1=OP.add
            )
            e_lam = pp.tile([H, W], f32)
            nc.scalar.activation(e_lam[:], lmo[:], AF.Exp)

            nc.vector.tensor_copy(s_full[0:VC, :], s_ps[:])
            nc.sync.dma_start(s_full[VC:H, 0:HPX], s_px[0:64, :])
            nc.sync.dma_start(s_full[VC:H, HPX:W], s_px[64:128, :])

            rs = pp.tile([H, W], f32)
            nc.vector.reciprocal_approx_fast(rs[:], s_full[:])
            lnb = pp.tile([H, W], f32)
            nc.vector.tensor_copy(lnb[:], s_full[:].bitcast(i32))
            ln_s = pp.tile([H, W], f32)
            nc.gpsimd.tensor_scalar(
                ln_s[:],
                lnb[:],
                LN2 / (2.0**23),
                (SLN - 127.0) * LN2,
                op0=OP.mult,
                op1=OP.add,
            )
            p = pp.tile([H, W], f32)
            nc.vector.tensor_tensor(p[:], e_lam[:], rs[:], op=OP.mult)
            logp = pp.tile([H, W], f32)
            nc.gpsimd.tensor_tensor(logp[:], lmo[:], ln_s[:], op=OP.subtract)
            omm = pp.tile([H, W], f32)
            nc.vector.tensor_scalar(
                omm[:], p[:], -1.0, 1.0, op0=OP.mult, op1=OP.add
            )
            sq = pp.tile([H, W], f32)
            nc.vector.tensor_tensor(sq[:], omm[:], omm[:], op=OP.mult)
            t1 = pp.tile([H, W], f32)
            nc.vector.tensor_tensor(t1[:], sq[:], logp[:], op=OP.mult)
            wl = pp.tile([H, W], bf16)
            nc.vector.scalar_tensor_tensor(
                wl[:], wgt[:], 1.0, t1[:], op0=OP.add, op1=OP.mult
            )
            osum_ps = op_.tile([1, W], f32)
            nc.tensor.matmul(osum_ps[:], ones96[:], wl[:], start=True, stop=True)
            osum = pp.tile([1, 1], f32)
            nc.vector.tensor_reduce(
                osum[:], osum_ps[:], axis=mybir.AxisListType.X, op=OP.add
            )
            nc.sync.dma_start(out_d[:], osum[:])
            if dbg:
                nc.sync.dma_start(dbg_m[:], mstar[:])
                nc.sync.dma_start(dbg_s[:], s_full[:])

    nc.compile()
    return nc


def _bin_of(depth):
    d = np.float32(depth)
    bin_size = np.float32(2.0 * (DEPTH_MAX - DEPTH_MIN) / (NUM_BINS * (1 + NUM_BINS)))
    idx = np.float32(-0.5) + np.float32(0.5) * np.sqrt(
        np.float32(1.0) + np.float32(8.0) * (d - np.float32(DEPTH_MIN)) / bin_size
    )
    bad = (idx < 0) | (idx > NUM_BINS) | ~np.isfinite(idx)
    idx = np.where(bad, np.float32(NUM_BINS), idx)
    return np.rint(idx).astype(np.int32)


def _host_prep(depth_logits, gt_boxes2d, num_gt_per_img, gt_center_depth):
    import ml_dtypes

    n = int(num_gt_per_img)
    boxes = np.asarray(gt_boxes2d, np.float32).reshape(B, n, 4)
    depths = np.asarray(gt_center_depth, np.float32).reshape(B, n)
    logits_f8 = np.asarray(depth_logits, np.float32).astype(ml_dtypes.float8_e4m3fn)

    diag32 = np.zeros((C, VC * VC), np.float32)
    for r in range(VC):
        diag32[:, VC * r + r] = 1.0
    diag32 = diag32.astype(ml_dtypes.float8_e4m3fn)
    ident96 = np.eye(H, dtype=np.float32).astype(ml_dtypes.float8_e4m3fn)

    us = np.arange(W)
    vs = np.arange(H, dtype=np.float32)

    in_maps = []
    for i in range(B):
        lg = logits_f8[i]
        lch = np.ascontiguousarray(lg[:, :VC, :].reshape(C, FC))
        blk = lg[:, VC:, :]
        lpx = np.empty((128, PXCOL), ml_dtypes.float8_e4m3fn)
        lpx[0:64] = blk[:, :, 0:HPX].transpose(1, 2, 0).reshape(64, PXCOL)
        lpx[64:128] = blk[:, :, HPX:W].transpose(1, 2, 0).reshape(64, PXCOL)

        bins = _bin_of(depths[i])
        order = np.argsort(bins, kind="stable")
        u1 = np.floor(boxes[i, order, 0]).astype(int)
        v1 = boxes[i, order, 1]
        u2 = np.ceil(boxes[i, order, 2]).astype(int)
        v2 = boxes[i, order, 3]
        cbins = bins[order]
        cand = np.full((W, K), NUM_BINS, np.int32)
        onehot = np.zeros((17, W * K), np.float32)
        cval = np.full((W * K), 2.0 * BIG, np.float32)
        for u in range(W):
            cov = [r for r in range(n) if u1[r] <= u < u2[r]]
            assert len(cov) <= K - 1, f"K too small: {len(cov)}"
            slots = [(16, NUM_BINS)] + [(r, cbins[r]) for r in cov]
            for s, (r, b) in enumerate(slots):
                cand[u, s] = b
                onehot[r, u * K + s] = 1.0
                cval[u * K + s] = BIG + STRIDE * r + OFF
        lg2 = lg.transpose(1, 2, 0)
        lgat = np.take_along_axis(
            lg2, np.broadcast_to(cand[None, :, :], (H, W, K)), axis=2
        ).reshape(H, KCOL)
        lgat = np.ascontiguousarray(lgat)

        rowm = (vs[None, :] >= np.floor(v1)[:, None]) & (
            vs[None, :] < np.ceil(v2)[:, None]
        )
        w18 = np.zeros((18, H), np.float32)
        w18[:16] = -BIG * rowm.astype(np.float32)
        w18[16] = -BIG
        w18[17] = 1.0
        bdc = np.concatenate([onehot, cval[None, :]], axis=0)
        bdcw = np.concatenate([bdc, w18], axis=1).astype(ml_dtypes.bfloat16)

        in_maps.append(
            {
                "lch": lch,
                "lpx": lpx,
                "lgat": lgat,
                "bdcw": bdcw,
                "diag32": diag32,
                "ident96": ident96,
            }
        )
    return in_maps


def get_program():
    global _PROG
    if _PROG is None:
        _PROG = _build_program()
    return _PROG


def kernel(depth_logits, gt_boxes2d, num_gt_per_img, gt_center_depth, _trace=False):
    from concourse import bass_utils

    nc = get_program()
    in_maps = _host_prep(depth_logits, gt_boxes2d, num_gt_per_img, gt_center_depth)
    res = bass_utils.run_bass_kernel_spmd(
        nc, in_maps, core_ids=list(range(B)), trace=_trace
    )
    total = np.float64(0.0)
    for r in res.results:
        total += np.float64(r["out"].astype(np.float64).sum())
    loss = np.float32(-ALPHA * total / (B * H * W))
    if _trace:
        kernel._last_results = res
    return np.asarray(loss, dtype=np.float32)


# revision 13
# speedup vs baseline: 1.2264x; 1.0550x over previous
import sys

sys.path.insert(0, "/opt/trn_rl_repo")

import numpy as np

B, C, H, W = 8, 81, 96, 320
F = H * W
NBOX = 16
ALPHA = 0.25
DEPTH_MIN, DEPTH_MAX, NUM_BINS = 0.001, 60.0, 80

STRIDE = 32.0
OFF = 16.0
BIG = 1024.0
K = 7
KCOL = W * K

VC = 32
FC = VC * W
HPX = W // 2
PXCOL = (H - VC) * W * C // 128

NCH = 4
NPX = 4
CHCOL = FC // NCH
PXCHUNK = PXCOL // NPX
GPC = PXCHUNK // C
TPC = HPX // NPX

CH_SPLIT = (1824, 608, 128)
PX_SPLIT_G = (22, 15, 3)

A_SCH = 1024.0 * np.float64(np.log2(np.e))
B_SCH = 15360.0 - 60.0
LN2 = float(np.log(2.0))
SLN = 0.0435

_PROG = None


def _build_program():
    from concourse import bacc, tile, mybir

    f32 = mybir.dt.float32
    f16 = mybir.dt.float16
    bf16 = mybir.dt.bfloat16
    i16 = mybir.dt.int16
    i32 = mybir.dt.int32
    f8 = mybir.dt.float8e4
    AF = mybir.ActivationFunctionType
    OP = mybir.AluOpType

    nc = bacc.Bacc(
        "TRN2",
        target_bir_lowering=False,
        debug=False,
        enable_asserts=False,
    )

    lch_d = nc.dram_tensor("lch", [C, FC], f8, kind="ExternalInput")
    lpx_d = nc.dram_tensor("lpx", [128, PXCOL], f8, kind="ExternalInput")
    lgat_d = nc.dram_tensor("lgat", [H, KCOL], f8, kind="ExternalInput")
    bdcw_d = nc.dram_tensor("bdcw", [18, KCOL + H], bf16, kind="ExternalInput")
    consts8_d = nc.dram_tensor("consts8", [H, VC * VC + H], f8, kind="ExternalInput")
    out_d = nc.dram_tensor("out", [1, 1], f32, kind="ExternalOutput")

    import os

    dbg = os.environ.get("KERNEL_DEBUG") == "1"
    if dbg:
        dbg_m = nc.dram_tensor("dbg_m", [H, W], f32, kind="ExternalOutput")
        dbg_s = nc.dram_tensor("dbg_s", [H, W], f32, kind="ExternalOutput")

    with tile.TileContext(nc) as tc:
        with (
            tc.tile_pool(name="persist", bufs=1) as pp,
            tc.tile_pool(name="tree", bufs=2) as tp,
            tc.tile_pool(name="spsum", bufs=1, space="PSUM") as sp,
            tc.tile_pool(name="ppsum", bufs=1, space="PSUM") as qp,
            tc.tile_pool(name="opsum", bufs=1, space="PSUM") as op_,
        ):
            lch = pp.tile([C, FC], f8)
            lpx = pp.tile([128, PXCOL], f8)
            ec = pp.tile([C, FC], f16)
            epx = pp.tile([128, PXCOL], f16)
            lgat = pp.tile([H, KCOL], f8)
            bdcw = pp.tile([18, KCOL + H], bf16)
            consts8 = pp.tile([H, VC * VC + H], f8)
            s_px = pp.tile([128, HPX], f32)
            s_full = pp.tile([H, W], f32)
            ones96 = pp.tile([H, 1], bf16)
            nc.vector.memset(ones96[:], 1.0)
            diag32 = consts8[0:C, 0 : VC * VC]
            ident96 = consts8[:, VC * VC : VC * VC + H]

            def dma_lch(ring, k):
                ring.dma_start(
                    lch[:, k * CHCOL : (k + 1) * CHCOL],
                    lch_d[:, k * CHCOL : (k + 1) * CHCOL],
                )

            def dma_lpx(ring, k):
                ring.dma_start(
                    lpx[:, k * PXCHUNK : (k + 1) * PXCHUNK],
                    lpx_d[:, k * PXCHUNK : (k + 1) * PXCHUNK],
                )

            dma_lch(nc.sync, 0)
            dma_lpx(nc.sync, 2)
            dma_lch(nc.sync, 3)
            dma_lpx(nc.scalar, 0)
            dma_lch(nc.scalar, 1)
            dma_lpx(nc.scalar, 3)
            nc.gpsimd.dma_start(consts8[:], consts8_d[:])
            nc.gpsimd.dma_start(bdcw[:], bdcw_d[:])
            nc.gpsimd.dma_start(lpx[:, PXCHUNK : 2 * PXCHUNK], lpx_d[:, PXCHUNK : 2 * PXCHUNK])
            nc.gpsimd.dma_start(lgat[:], lgat_d[:])
            dma_lch(nc.gpsimd, 2)

            s_ps = sp.tile([VC, W], f32)
            pen = qp.tile([H, KCOL], f32)
            mstar = pp.tile([H, W], f32)

            w18 = bdcw[:, KCOL : KCOL + H]
            for c0 in range(0, KCOL, 512):
                cn = min(512, KCOL - c0)
                nc.tensor.matmul(
                    pen[:, c0 : c0 + cn],
                    w18,
                    bdcw[:, c0 : c0 + cn],
                    start=True,
                    stop=False,
                )

            def exp_split(dst, src, base, widths):
                a, p, d = widths
                c0 = base
                nc.scalar.activation(dst[:, c0 : c0 + a], src[:, c0 : c0 + a], AF.Exp)
                c0 += a
                nc.gpsimd.tensor_scalar(
                    dst[:, c0 : c0 + p].bitcast(i16),
                    src[:, c0 : c0 + p],
                    A_SCH,
                    B_SCH,
                    op0=OP.mult,
                    op1=OP.add,
                )
                c0 += p
                if d:
                    nc.vector.tensor_scalar(
                        dst[:, c0 : c0 + d].bitcast(i16),
                        src[:, c0 : c0 + d],
                        A_SCH,
                        B_SCH,
                        op0=OP.mult,
                        op1=OP.add,
                    )

            sfull2 = s_full[VC:H, :].rearrange("p (h t) -> p h t", h=2)

            rows_per_chunk = VC // NCH
            for k in range(max(NCH, NPX)):
                if k < NPX:
                    base = k * PXCHUNK
                    ga, gp, gd = PX_SPLIT_G
                    exp_split(epx, lpx, base, (ga * C, gp * C, gd * C))
                if k < NCH:
                    exp_split(ec, lch, k * CHCOL, CH_SPLIT)
                    for r in range(rows_per_chunk):
                        v = k * rows_per_chunk + r
                        nc.tensor.matmul(
                            s_ps[:],
                            diag32[:, VC * v : VC * (v + 1)],
                            ec[:, v * W : (v + 1) * W],
                            start=(v == 0),
                            stop=(v == VC - 1),
                        )
                if k < NPX:
                    base = k * PXCHUNK
                    g3 = epx[:, base : base + PXCHUNK].rearrange(
                        "p (g c) -> p g c", c=C
                    )
                    tb = tp.tile([128, GPC * 40], f16, tag="tb")
                    tc_ = tp.tile([128, GPC * 21], f16, tag="tc")
                    b3 = tb[:].rearrange("p (g c) -> p g c", c=40)
                    c3 = tc_[:].rearrange("p (g c) -> p g c", c=21)
                    TT = nc.vector.tensor_tensor
                    TT(b3, g3[:, :, 0:40], g3[:, :, 40:80], op=OP.add)
                    nc.vector.tensor_copy(c3[:, :, 20:21], g3[:, :, 80:81])
                    TT(c3[:, :, 0:20], b3[:, :, 0:20], b3[:, :, 20:40], op=OP.add)
                    nc.vector.tensor_reduce(
                        s_px[:, k * TPC : (k + 1) * TPC],
                        c3,
                        axis=mybir.AxisListType.X,
                        op=OP.add,
                    )
                    nc.sync.dma_start(
                        sfull2[:, :, k * TPC : (k + 1) * TPC],
                        s_px[:, k * TPC : (k + 1) * TPC],
                    )
                if k == 1:
                    for c0 in range(0, KCOL, 512):
                        cn = min(512, KCOL - c0)
                        nc.tensor.matmul(
                            pen[:, c0 : c0 + cn],
                            ident96[:],
                            lgat[:, c0 : c0 + cn],
                            start=False,
                            stop=True,
                        )

            nc.vector.tensor_reduce(
                mstar[:],
                pen[:].rearrange("v (u k) -> v u k", k=K),
                axis=mybir.AxisListType.X,
                op=OP.min,
            )
            r_i = pp.tile([H, W], i32)
            nc.vector.tensor_scalar(
                r_i[:], mstar[:], 1.0 / STRIDE, -0.25, op0=OP.mult, op1=OP.add
            )
            r_f = pp.tile([H, W], f32)
            nc.vector.tensor_copy(r_f[:], r_i[:])
            lam = pp.tile([H, W], f32)
            nc.vector.scalar_tensor_tensor(
                lam[:], r_f[:], -STRIDE, mstar[:], op0=OP.mult, op1=OP.add
            )
            wgt = pp.tile([H, W], f32)
            nc.gpsimd.tensor_scalar(
                wgt[:], mstar[:], STRIDE * NBOX, 12.0, op0=OP.is_lt, op1=OP.mult
            )
            wq = pp.tile([H, W], f32)
            nc.gpsimd.tensor_scalar(
                wq[:], wgt[:], 1.0, 1.0, op0=OP.mult, op1=OP.add
            )
            lmo = pp.tile([H, W], f32)
            nc.gpsimd.tensor_scalar(
                lmo[:], lam[:], 1.0, -OFF, op0=OP.mult, op1=OP.add
            )
            e_lam = pp.tile([H, W], f32)
            nc.scalar.activation(e_lam[:], lmo[:], AF.Exp)

            nc.vector.tensor_copy(s_full[0:VC, :], s_ps[:])

            rs = pp.tile([H, W], f32)
            nc.vector.reciprocal_approx_fast(rs[:], s_full[:])
            lnb = pp.tile([H, W], f32)
            nc.vector.tensor_copy(lnb[:], s_full[:].bitcast(i32))
            ln_s = pp.tile([H, W], f32)
            nc.gpsimd.tensor_scalar(
                ln_s[:],
                lnb[:],
                LN2 / (2.0**23),
                (SLN - 127.0) * LN2,
                op0=OP.mult,
                op1=OP.add,
            )
            logp = pp.tile([H, W], f32)
            nc.gpsimd.tensor_tensor(logp[:], lmo[:], ln_s[:], op=OP.subtract)
            p = pp.tile([H, W], f32)
            nc.vector.tensor_tensor(p[:], e_lam[:], rs[:], op=OP.mult)
            omm = pp.tile([H, W], f32)
            nc.vector.tensor_scalar(
                omm[:], p[:], -1.0, 1.0, op0=OP.mult, op1=OP.add
            )
            sq = pp.tile([H, W], f32)
            nc.vector.tensor_tensor(sq[:], omm[:], omm[:], op=OP.mult)
            t1 = pp.tile([H, W], f32)
            nc.vector.tensor_tensor(t1[:], sq[:], logp[:], op=OP.mult)
            wl = pp.tile([H, W], bf16)
            nc.vector.tensor_tensor(wl[:], wq[:], t1[:], op=OP.mult)
            osum_ps = op_.tile([1, W], f32)
            nc.tensor.matmul(osum_ps[:], ones96[:], wl[:], start=True, stop=True)
            osum = pp.tile([1, 1], f32)
            nc.vector.tensor_reduce(
                osum[:], osum_ps[:], axis=mybir.AxisListType.X, op=OP.add
            )
            nc.sync.dma_start(out_d[:], osum[:])
            if dbg:
                nc.sync.dma_start(dbg_m[:], mstar[:])
                nc.sync.dma_start(dbg_s[:], s_full[:])

    nc.compile()
    return nc


def _bin_of(depth):
    d = np.float32(depth)
    bin_size = np.float32(2.0 * (DEPTH_MAX - DEPTH_MIN) / (NUM_BINS * (1 + NUM_BINS)))
    idx = np.float32(-0.5) + np.float32(0.5) * np.sqrt(
        np.float32(1.0) + np.float32(8.0) * (d - np.float32(DEPTH_MIN)) / bin_size
    )
    bad = (idx < 0) | (idx > NUM_BINS) | ~np.isfinite(idx)
    idx = np.where(bad, np.float32(NUM_BINS), idx)
    return np.rint(idx).astype(np.int32)


def _host_prep(depth_logits, gt_boxes2d, num_gt_per_img, gt_center_depth):
    import ml_dtypes

    n = int(num_gt_per_img)
    boxes = np.asarray(gt_boxes2d, np.float32).reshape(B, n, 4)
    depths = np.asarray(gt_center_depth, np.float32).reshape(B, n)
    logits_f8 = np.asarray(depth_logits, np.float32).astype(ml_dtypes.float8_e4m3fn)

    consts8 = np.zeros((H, VC * VC + H), np.float32)
    for r in range(VC):
        consts8[:C, VC * r + r] = 1.0
    consts8[:, VC * VC :] = np.eye(H, dtype=np.float32)
    consts8 = consts8.astype(ml_dtypes.float8_e4m3fn)

    vs = np.arange(H, dtype=np.float32)

    in_maps = []
    for i in range(B):
        lg = logits_f8[i]
        lch = np.ascontiguousarray(lg[:, :VC, :].reshape(C, FC))
        blk = lg[:, VC:, :]
        t2 = blk.reshape(C, H - VC, 2, HPX).transpose(1, 2, 3, 0)
        lpx = np.ascontiguousarray(t2.reshape(128, PXCOL))

        bins = _bin_of(depths[i])
        order = np.argsort(bins, kind="stable")
        u1 = np.floor(boxes[i, order, 0]).astype(int)
        v1 = boxes[i, order, 1]
        u2 = np.ceil(boxes[i, order, 2]).astype(int)
        v2 = boxes[i, order, 3]
        cbins = bins[order]
        cand = np.full((W, K), NUM_BINS, np.int32)
        onehot = np.zeros((17, W * K), np.float32)
        cval = np.full((W * K), 2.0 * BIG, np.float32)
        for u in range(W):
            cov = [r for r in range(n) if u1[r] <= u < u2[r]]
            assert len(cov) <= K - 1, f"K too small: {len(cov)}"
            slots = [(16, NUM_BINS)] + [(r, cbins[r]) for r in cov]
            for s, (r, b) in enumerate(slots):
                cand[u, s] = b
                onehot[r, u * K + s] = 1.0
                cval[u * K + s] = BIG + STRIDE * r + OFF
        lg2 = lg.transpose(1, 2, 0)
        lgat = np.take_along_axis(
            lg2, np.broadcast_to(cand[None, :, :], (H, W, K)), axis=2
        ).reshape(H, KCOL)
        lgat = np.ascontiguousarray(lgat)

        rowm = (vs[None, :] >= np.floor(v1)[:, None]) & (
            vs[None, :] < np.ceil(v2)[:, None]
        )
        w18 = np.zeros((18, H), np.float32)
        w18[:16] = -BIG * rowm.astype(np.float32)
        w18[16] = -BIG
        w18[17] = 1.0
        bdc = np.concatenate([onehot, cval[None, :]], axis=0)
        bdcw = np.concatenate([bdc, w18], axis=1).astype(ml_dtypes.bfloat16)

        in_maps.append(
            {
                "lch": lch,
                "lpx": lpx,
                "lgat": lgat,
                "bdcw": bdcw,
                "consts8": consts8,
            }
        )
    return in_maps


def get_program():
    global _PROG
    if _PROG is None:
        _PROG = _build_program()
    return _PROG


def kernel(depth_logits, gt_boxes2d, num_gt_per_img, gt_center_depth, _trace=False):
    from concourse import bass_utils

    nc = get_program()
    in_maps = _host_prep(depth_logits, gt_boxes2d, num_gt_per_img, gt_center_depth)
    res = bass_utils.run_bass_kernel_spmd(
        nc, in_maps, core_ids=list(range(B)), trace=_trace
    )
    total = np.float64(0.0)
    for r in res.results:
        total += np.float64(r["out"].astype(np.float64).sum())
    loss = np.float32(-ALPHA * total / (B * H * W))
    if _trace:
        kernel._last_results = res
    return np.asarray(loss, dtype=np.float32)
